# revision 1
# baseline (speedup 1.0000x reference)
"""Trainium2 Bass kernel for nn_AttentionBlock (B=8, S=2048, D=1024).

Reference computation (per batch element b):
    q = x @ Wq + bq ; k = x @ Wk + bk ; v = x @ Wv + bv
    scores = (q @ k^T) / sqrt(1024)
    attn = softmax(scores, axis=QUERY)          # axis=1 of [B, S_q, S_k]!
    out = attn @ v

Sharding: pure data-parallel — batch element b runs on NeuronCore b.

Device algorithm (bf16 matmul inputs, fp32 PSUM accumulation):
  - weight folding (host, fp64, recomputed from the actual inputs each
    call): A = Wq Wk^T, u = Wq bk, w = Wk bq, c = bq.bk, so that
        scores_raw[i, j] = x_i A x_j^T + x.u|_i + x.w|_j + c
    This removes the separate q/k projections (two 1024^3 matmuls) in
    favour of one (y = x A) plus cheap rank-1 corrections.
  - host supplies x^T (bf16, PE tile layout), so every projection is a
    plain `out = lhsT.T @ rhs` with the contraction (emb) on partitions.
  - scores are computed TRANSPOSED: sT[j, i], so the softmax reduction
    axis (i = query) is the free axis.  The scaled scores lie in ~[-3, 3]
    for this data distribution (x ~ N(0,1), W ~ U(+-1/32) keep them ~40
    sigma below exp overflow), so softmax needs no max subtraction.
  - E~ = exp(scale*(core + r2_j + c)) via one ScalarE pass (r2+c as the
    per-partition activation bias).  The query-side factor
    g_i = exp(scale*r1_i) is applied only (a) inside the weighted
    Z_j = sum_i E~[j,i] g_i (DVE mul into a scratch tile + reduce_sum)
    and (b) as a per-partition scale of the final output tiles — so E~
    itself is only rounded to bf16 once.  1/Z is folded into v rows:
    out[i, :] = g_i * sum_j E~^T[j, i] * (v[j, :] / Z_j).
"""

import numpy as np
import ml_dtypes

S = 2048          # sequence length
E = 1024          # emb dim == att dim
P = 128           # partitions
NS = S // P       # 16 sequence tiles
NE = E // P       # 8 emb tiles
NCORES = 8
SCALE = 1.0 / 32.0  # 1/sqrt(1024)

_BUILT = {}


def _build(reps=1):
    """Construct the Bass program (same NEFF for all 8 cores).

    reps>1 emits the body multiple times back-to-back (benchmarking only:
    wall(K) - wall(1) = (K-1) * body time, cancelling launch/transfer
    overhead that dominates wall measurements through the axon tunnel).
    """
    import concourse.tile as tile
    import concourse.mybir as mybir
    from concourse import bacc

    nc = bacc.Bacc("TRN2", target_bir_lowering=False, debug=False)

    f32 = mybir.dt.float32
    bf16 = mybir.dt.bfloat16

    xT_d = nc.dram_tensor("xT", [P, NE, S], bf16, kind="ExternalInput").ap()
    a_d = nc.dram_tensor("A", [P, NE, E], bf16, kind="ExternalInput").ap()
    wv_d = nc.dram_tensor("Wv", [P, NE, E], bf16, kind="ExternalInput").ap()
    uw_d = nc.dram_tensor("uw", [P, NE, 2], bf16, kind="ExternalInput").ap()
    cc_d = nc.dram_tensor("cc", [P, 1], f32, kind="ExternalInput").ap()
    bv_d = nc.dram_tensor("bv", [P, E], bf16, kind="ExternalInput").ap()
    out_d = nc.dram_tensor("out", [S, E], f32, kind="ExternalOutput").ap()
    r2_d = nc.dram_tensor("r2scratch", [2, S], f32).ap()  # internal

    with tile.TileContext(nc) as tc:
        for _ in range(reps):
            _emit_body(nc, tc, xT_d, a_d, wv_d, uw_d, cc_d, bv_d, out_d, r2_d)

    nc.compile()
    return nc


def _emit_body(nc, tc, xT_d, a_d, wv_d, uw_d, cc_d, bv_d, out_d, r2_d):
    from contextlib import ExitStack
    import concourse.mybir as mybir

    f32 = mybir.dt.float32
    bf16 = mybir.dt.bfloat16
    Act = mybir.ActivationFunctionType

    with ExitStack() as ctx:
        const_p = ctx.enter_context(tc.tile_pool(name="const", bufs=1))
        bv_t = const_p.tile([P, E], bf16)
        cc_t = const_p.tile([P, 1], f32)
        g1_t = const_p.tile([1, S], bf16)
        gf_t = const_p.tile([P, S], bf16)
        rr_t = const_p.tile([2, S], f32)
        r1T_t = const_p.tile([P, NS], f32)
        gT_t = const_p.tile([P, NS], f32)
        r2T_t = const_p.tile([P, NS], f32)
        bias_t = const_p.tile([P, NS], f32)
        zz = const_p.tile([P, NS], f32)
        zr = const_p.tile([P, NS], f32)

        yT_p = ctx.enter_context(tc.tile_pool(name="yT", bufs=1))
        yT = yT_p.tile([P, NE, S], bf16)
        v_p = ctx.enter_context(tc.tile_pool(name="v", bufs=1))
        v_t = v_p.tile([P, NS, E], bf16)
        xT_p = ctx.enter_context(tc.tile_pool(name="xT", bufs=NE + 1))

        # single PSUM pool for the whole kernel: 4 slots of [P, E]
        # (2 banks each) -> 4 accumulation chains in flight, single-copy
        # slot release, no pool-handoff bubbles between phases
        psv = ctx.enter_context(tc.tile_pool(name="psv", bufs=4,
                                             space="PSUM"))

        with ExitStack() as ph1:
            w_p = ph1.enter_context(tc.tile_pool(name="w", bufs=1))
            # interleave xT / Wv chunk DMAs so the first v-matmul's
            # dependencies land first; chunk 0 is halved so the first
            # matmul can start as early as possible
            xts, wvs, ats = [], [], []
            wv_t = w_p.tile([P, NE, E], bf16, tag="wv")
            at_t = w_p.tile([P, NE, E], bf16, tag="at")
            wvs = [wv_t[:, e, :] for e in range(NE)]
            ats = [at_t[:, e, :] for e in range(NE)]
            xt0 = xT_p.tile([P, S], bf16, tag="xt")
            nc.sync.dma_start(xt0[:, 0:S // 2], xT_d[:, 0, 0:S // 2])
            nc.sync.dma_start(wv_t[:, 0, 0:E // 2], wv_d[:, 0, 0:E // 2])
            nc.sync.dma_start(xt0[:, S // 2:S], xT_d[:, 0, S // 2:S])
            nc.sync.dma_start(wv_t[:, 0, E // 2:E], wv_d[:, 0, E // 2:E])
            xts.append(xt0)
            for e in range(1, NE):
                t = xT_p.tile([P, S], bf16, tag="xt")
                nc.sync.dma_start(t[:], xT_d[:, e, :])
                xts.append(t)
                if e in (1, 3, 5):
                    # Wv chunks land two at a time (chunk 7 alone): fewer
                    # slots on the serial DMA-issue track
                    nc.sync.dma_start(wv_t[:, e:e + 2, :], wv_d[:, e:e + 2, :])
                elif e == 7:
                    nc.sync.dma_start(wv_t[:, 7:8, :], wv_d[:, 7:8, :])
            nc.sync.dma_start(cc_t[:], cc_d)
            nc.sync.dma_start(bv_t[:], bv_d)
            uw_t = w_p.tile([P, NE, 2], bf16, tag="uw")
            nc.sync.dma_start(uw_t[:], uw_d)
            for e2 in range(0, NE, 2):
                nc.sync.dma_start(at_t[:, e2:e2 + 2, :], a_d[:, e2:e2 + 2, :])

            # ---- v = x @ Wv + bv : v_t[:, j, :] = v[j*P:(j+1)*P, :] ----
            for j in range(NS):
                pv = psv.tile([P, E], f32, tag="pv")
                for e in range(NE):
                    lhsT = xts[e][:, j * P:(j + 1) * P]
                    for c in range(2):
                        cs = slice(c * 512, (c + 1) * 512)
                        nc.tensor.matmul(pv[:, cs], lhsT, wvs[e][:, cs],
                                         start=(e == 0), stop=(e == NE - 1))
                # fused bias add + cast during PSUM -> SBUF
                nc.vector.tensor_tensor(v_t[:, j, :], pv[:, :], bv_t[:],
                                        op=mybir.AluOpType.add)

            # ---- rank-1 terms: r1[i] = x_i.u ; r2[j] = x_j.w ----
            prs = []
            for h in range(2):
                pr = psv.tile([2, E], f32, tag="pv")
                for e in range(NE):
                    lhsT = uw_t[:, e, :]
                    for c in range(2):
                        cs = slice(c * 512, (c + 1) * 512)
                        nc.tensor.matmul(pr[:, cs], lhsT,
                                         xts[e][:, h * E + c * 512:
                                                h * E + (c + 1) * 512],
                                         start=(e == 0), stop=(e == NE - 1))
                prs.append(pr)
            for h in range(2):
                nc.vector.tensor_copy(rr_t[:, h * E:(h + 1) * E], prs[h][0:2, :])
            # g[i] = exp(scale * r1_i), broadcast to all partitions (used
            # only for the weighted Z; the output itself is scaled by gT)
            nc.scalar.activation(g1_t[:], rr_t[0:1, :], func=Act.Exp,
                                 scale=SCALE)
            nc.gpsimd.partition_broadcast(gf_t[:], g1_t[:])
            # transpose r1, r2 [1, S] -> [P, NS] via DRAM round trip
            nc.sync.dma_start(r2_d[:, :], rr_t[0:2, :])
            nc.sync.dma_start(
                r1T_t[:], r2_d[0:1, :].rearrange("a (t p) -> (a p) t", p=P))
            nc.sync.dma_start(
                r2T_t[:], r2_d[1:2, :].rearrange("a (t p) -> (a p) t", p=P))
            nc.scalar.activation(gT_t[:], r1T_t[:], func=Act.Exp, scale=SCALE)
            # exp bias: scale * (r2_j + c), per partition for each j-tile
            nc.vector.tensor_scalar(bias_t[:], r2T_t[:], cc_t[:, 0:1], SCALE,
                                    op0=mybir.AluOpType.add,
                                    op1=mybir.AluOpType.mult)

            # ---- yT[:, d, :] = (x @ A).T  d-tile rows (two halves) ----
            for d in range(NE):
                for h in range(2):
                    pq = psv.tile([P, E], f32, tag="pv")
                    for e in range(NE):
                        lhsT = ats[e][:, d * P:(d + 1) * P]
                        for c in range(2):
                            cs = slice(h * E + c * 512, h * E + (c + 1) * 512)
                            nc.tensor.matmul(pq[:, c * 512:(c + 1) * 512],
                                             lhsT, xts[e][:, cs],
                                             start=(e == 0), stop=(e == NE - 1))
                    nc.scalar.copy(yT[:, d, h * E:(h + 1) * E], pq[:, :])

        # ---- scoresT + softmax-over-query + fold 1/Z into v ----
        Et_p = ctx.enter_context(tc.tile_pool(name="Et", bufs=1))
        Et = Et_p.tile([P, NS, S], bf16)
        tmp_p = ctx.enter_context(tc.tile_pool(name="tmp", bufs=1))
        for j in range(NS):
            for h in range(2):
                pss = psv.tile([P, E], f32, tag="pv")
                for d in range(NE):
                    lhsT = xts[d][:, j * P:(j + 1) * P]
                    for c in range(2):
                        cs = slice(h * E + c * 512, h * E + (c + 1) * 512)
                        nc.tensor.matmul(pss[:, c * 512:(c + 1) * 512],
                                         lhsT, yT[:, d, cs],
                                         start=(d == 0), stop=(d == NE - 1))
                nc.scalar.activation(Et[:, j, h * E:(h + 1) * E], pss[:, :],
                                     func=Act.Exp, scale=SCALE,
                                     bias=bias_t[:, j:j + 1])
            # Z_j = sum_i E~[j,i] * g_i  (throwaway product; E~ itself stays
            # single-rounded — g is applied per-partition on the output)
            tmp = tmp_p.tile([P, S], bf16, tag="tmp")
            nc.vector.tensor_mul(tmp[:], Et[:, j, :], gf_t[:])
            nc.vector.reduce_sum(zz[:, j:j + 1], tmp[:],
                                 axis=mybir.AxisListType.X)
            nc.vector.reciprocal(zr[:, j:j + 1], zz[:, j:j + 1])
            nc.vector.tensor_scalar_mul(v_t[:, j, :], v_t[:, j, :],
                                        zr[:, j:j + 1])

        # ---- out[i, :] = sum_j E^T[j, i-tile] . v'[j] ----
        ost_p = ctx.enter_context(tc.tile_pool(name="ost", bufs=3))
        for i in range(NS - 1):
            po = psv.tile([P, E], f32, tag="pv")
            for j in range(NS):
                lhsT = Et[:, j, i * P:(i + 1) * P]
                for c in range(2):
                    cs = slice(c * 512, (c + 1) * 512)
                    nc.tensor.matmul(po[:, cs], lhsT, v_t[:, j, cs],
                                     start=(j == 0), stop=(j == NS - 1))
            ob = ost_p.tile([P, E], f32, tag="ost")
            # the two gT-scaled PSUM->SBUF copies run on different engines
            nc.scalar.activation(ob[:, 0:512], po[:, 0:512], func=Act.Copy,
                                 scale=gT_t[:, i:i + 1])
            nc.sync.dma_start(out_d[i * P:(i + 1) * P, 0:512], ob[:, 0:512])
            nc.vector.tensor_scalar_mul(ob[:, 512:1024], po[:, 512:1024],
                                        gT_t[:, i:i + 1])
            nc.sync.dma_start(out_d[i * P:(i + 1) * P, 512:1024],
                              ob[:, 512:1024])
        # last i-tile: two independent half-chains so the first half's
        # copy + DMA overlap the second half's matmuls (shorter tail)
        i = NS - 1
        ob = ost_p.tile([P, E], f32, tag="ost")
        for c in range(2):
            cs = slice(c * 512, (c + 1) * 512)
            ph = psv.tile([P, 512], f32, tag="pv")
            for j in range(NS):
                lhsT = Et[:, j, i * P:(i + 1) * P]
                nc.tensor.matmul(ph[:, :], lhsT, v_t[:, j, cs],
                                 start=(j == 0), stop=(j == NS - 1))
            if c == 0:
                nc.scalar.activation(ob[:, cs], ph[:, :], func=Act.Copy,
                                     scale=gT_t[:, i:i + 1])
            else:
                nc.vector.tensor_scalar_mul(ob[:, cs], ph[:, :],
                                            gT_t[:, i:i + 1])
            nc.sync.dma_start(out_d[i * P:(i + 1) * P, cs], ob[:, cs])


def _get_built():
    if "nc" not in _BUILT:
        _BUILT["nc"] = _build()
    return _BUILT["nc"]


def _tile_w(w):
    # [E, E] -> PE tile layout [P, NE, E]: [p, e, d] = W[e*P + p, d]
    return np.ascontiguousarray(
        np.asarray(w, dtype=np.float32).reshape(NE, P, E).transpose(1, 0, 2)
    ).astype(ml_dtypes.bfloat16)


def _make_in_maps(inputs):
    x = np.asarray(inputs["x_h"], dtype=np.float32)     # [8, S, E]
    Wq = np.asarray(inputs["Wq"], dtype=np.float64)
    bq = np.asarray(inputs["bq"], dtype=np.float64)
    Wk = np.asarray(inputs["Wk"], dtype=np.float64)
    bk = np.asarray(inputs["bk"], dtype=np.float64)
    Wv = np.asarray(inputs["Wv"], dtype=np.float32)
    bv = np.asarray(inputs["bv"], dtype=np.float32)

    # host weight folding (input-independent weight preprocessing, fp64)
    A = Wq @ Wk.T                                       # [E, E]
    u = Wq @ bk                                         # [E]
    w = Wk @ bq                                         # [E]
    c = float(bq @ bk)

    a_h = _tile_w(A)
    wv_h = _tile_w(Wv)
    uw_h = np.ascontiguousarray(
        np.stack([u.astype(np.float32).reshape(NE, P).T,
                  w.astype(np.float32).reshape(NE, P).T], axis=2)
    ).astype(ml_dtypes.bfloat16)                        # [P, NE, 2]
    cc_h = np.full((P, 1), c, dtype=np.float32)
    bv_h = np.ascontiguousarray(
        np.broadcast_to(bv.reshape(1, E), (P, E))).astype(ml_dtypes.bfloat16)

    in_maps = []
    for b in range(NCORES):
        # xT tile layout [P, NE, S]: [p, e, i] = x[b][i, e*P + p]
        xT_h = np.ascontiguousarray(
            x[b].T.reshape(NE, P, S).transpose(1, 0, 2)
        ).astype(ml_dtypes.bfloat16)
        in_maps.append({
            "xT": xT_h, "A": a_h, "Wv": wv_h, "uw": uw_h,
            "cc": cc_h, "bv": bv_h,
        })
    return in_maps


def kernel(**inputs):
    from concourse.bass_utils import run_bass_kernel_spmd

    nc = _get_built()
    in_maps = _make_in_maps(inputs)
    res = run_bass_kernel_spmd(nc, in_maps, list(range(NCORES)))
    out = np.stack([np.asarray(res.results[b]["out"], dtype=np.float32)
                    for b in range(NCORES)])
    return out



# revision 9
# speedup vs baseline: 1.6026x; 1.6026x over previous
"""Trainium2 Bass kernel for nn_AttentionBlock (B=8, S=2048, D=1024).

Reference computation (per batch element b):
    q = x @ Wq + bq ; k = x @ Wk + bk ; v = x @ Wv + bv
    scores = (q @ k^T) / sqrt(1024)
    attn = softmax(scores, axis=QUERY)          # axis=1 of [B, S_q, S_k]!
    out = attn @ v

Sharding: pure data-parallel — batch element b runs on NeuronCore b.

Device algorithm — fp8e4m3 matmuls in DoubleRow perf mode (two 128-deep
k-tiles per instruction at 0.5 cycles/output-column = 4x the bf16 MAC
rate), fp32 PSUM accumulation, out-free 512 per matmul.  Precision is
held inside the rel-err budget by hi/lo operand splitting
(a ~ fp8(a) + fp8(a - fp8(a))) on the paths where quantization error
passes straight through to the output:

  - weight folding (host, fp64): A = Wq Wk^T, u = Wq bk, w = Wk bq,
    c = bq.bk, so scores_raw[i,j] = x_i A x_j^T + r1_i + r2_j + c with
    r1 = x u, r2 = x w.  Removes the separate q/k projections.
  - host supplies xT in fp8 hi+lo ([P, e, i] PE tile layout), A*64 and
    Wv*32 in fp8 hi+lo (scaled to dodge fp8 subnormals; the scales
    fold into the exp argument / output copy).
  - v32 = x@(32Wv)+32bv: 3-term split -> bf16, then a STATIC fp8 hi/lo
    split (vHi/vLo) during phase 1 — no Z dependency.
  - y = x@(64A): 3-term split -> yT8 = fp8(64y).
  - scoresT[j,i] = xHi[j].yT8[i] + 64*r1_i; the rank-1 r1 row rides in
    as one extra DoubleRow matmul per chain (const 1/256 stationary x
    broadcast fp8 r1 row), so exp() emits the FULL softmax numerator
    E[j,i] and the activation accumulator produces Z_j for free.
  - The 1/Z_j softmax fold happens on the E side (keys = partitions of
    the Et tile): one DVE/Pool tensor_scalar pass casts
    Ep8 = fp8((E - 1) * 8192/Z_j), which simultaneously mean-centers E
    (3x smaller fp8 quantization error — no E-lo chain needed).
  - The dropped softmax mean sum_j v[j,:]/Z_j is restored exactly by
    two [16,512] psum row-chains over vHi/vLo: B1 with an exact
    const-4 stationary (carries the 1/Zbar part) and B2 with a small
    fp8 residual stationary dz = 64*(Zbar/Z_j - 1) (±2% values, so its
    quantization is second-order).  Their combination is broadcast and
    added during the final PSUM->SBUF copies.
  - out[i,:] = psumA/262144 + (16*B1 + B2)/(2048*Zbar).
"""

import numpy as np
import ml_dtypes

S = 2048          # sequence length
E = 1024          # emb dim == att dim
P = 128           # partitions
NS = S // P       # 16 sequence tiles
NE = E // P       # 8 emb tiles
NCORES = 8
SC = 1.0 / 2048.0  # exp scale on the x64-scaled psum: (1/32)*(1/64)

_BUILT = {}


def _build(reps=1):
    """Construct the Bass program (same NEFF for all 8 cores)."""
    import concourse.tile as tile
    import concourse.mybir as mybir
    from concourse import bacc

    nc = bacc.Bacc("TRN2", target_bir_lowering=False, debug=False)

    f32 = mybir.dt.float32
    bf16 = mybir.dt.bfloat16
    fp8 = mybir.dt.float8e4

    xhi_d = nc.dram_tensor("xhi", [P, NE, S], fp8, kind="ExternalInput").ap()
    xlo_d = nc.dram_tensor("xlo", [P, NE, S], fp8, kind="ExternalInput").ap()
    ahi_d = nc.dram_tensor("ahi", [P, NE, E], fp8, kind="ExternalInput").ap()
    alo_d = nc.dram_tensor("alo", [P, NE, E], fp8, kind="ExternalInput").ap()
    wvh_d = nc.dram_tensor("wvh", [P, NE, E], fp8, kind="ExternalInput").ap()
    wvl_d = nc.dram_tensor("wvl", [P, NE, E], fp8, kind="ExternalInput").ap()
    uw_d = nc.dram_tensor("uw", [P, NE, 16], fp8, kind="ExternalInput").ap()
    bv_d = nc.dram_tensor("bv", [P, E], bf16, kind="ExternalInput").ap()
    cc_d = nc.dram_tensor("cc", [P, 1], f32, kind="ExternalInput").ap()
    out_d = nc.dram_tensor("out", [S, E], f32, kind="ExternalOutput").ap()
    r2_d = nc.dram_tensor("r2scratch", [1, S], f32).ap()  # internal

    with tile.TileContext(nc) as tc:
        for _ in range(reps):
            _emit_body(nc, tc, xhi_d, xlo_d, ahi_d, alo_d, wvh_d, wvl_d,
                       uw_d, bv_d, cc_d, out_d, r2_d)

    nc.compile()
    return nc


def _emit_body(nc, tc, xhi_d, xlo_d, ahi_d, alo_d, wvh_d, wvl_d,
               uw_d, bv_d, cc_d, out_d, r2_d):
    from contextlib import ExitStack
    import concourse.mybir as mybir

    f32 = mybir.dt.float32
    bf16 = mybir.dt.bfloat16
    fp8 = mybir.dt.float8e4
    Act = mybir.ActivationFunctionType
    Alu = mybir.AluOpType
    DR = mybir.MatmulPerfMode.DoubleRow
    AxX = mybir.AxisListType.X
    AxC = mybir.AxisListType.C

    with ExitStack() as ctx:
        const_p = ctx.enter_context(tc.tile_pool(name="const", bufs=1))
        bv_t = const_p.tile([P, E], bf16)
        cc_t = const_p.tile([P, 1], f32)
        rr_t = const_p.tile([2, S], f32)
        r1b = const_p.tile([P, 2, S], fp8)
        r1r8 = const_p.tile([1, S], fp8)
        r2T = const_p.tile([P, NS], f32)
        bias_t = const_p.tile([P, NS], f32)
        zp = const_p.tile([P, 2 * NS], f32)   # per-(j,half) exp accums
        ztm = const_p.tile([P, NS], f32)
        sA = const_p.tile([P, NS], f32)       # 8192 / Z_j
        c0 = const_p.tile([P, 2, P], fp8)     # 1/256 stationary (r1 add)
        c4 = const_p.tile([P, 2, 16], fp8)    # 4.0 stationary (B1 row)
        c64 = const_p.tile([P, NS], f32)      # 64.0
        dzf = const_p.tile([P, NS], f32)
        dz8rep = const_p.tile([P, NS, 16], fp8)
        zrow = const_p.tile([P, 1], f32)
        zscal = const_p.tile([1, 1], f32)     # 2048 * Zbar
        zbi = const_p.tile([1, 1], f32)       # 1/(2048 Zbar)
        zq = const_p.tile([1, 1], f32)        # Zbar/128
        zbp = const_p.tile([P, 1], f32)
        rowT = const_p.tile([1, E], f32)
        rowS = const_p.tile([1, E], f32)
        rowF = const_p.tile([1, E], f32)
        mbc = const_p.tile([P, E], f32)
        nc.vector.memset(c0[:], 1.0 / 256.0)
        nc.vector.memset(c4[:], 4.0)
        nc.vector.memset(c64[:], 64.0)

        x_p = ctx.enter_context(tc.tile_pool(name="x", bufs=1))
        xhi = x_p.tile([P, NE, S], fp8)
        xlo = x_p.tile([P, NE, S], fp8)
        v_p = ctx.enter_context(tc.tile_pool(name="v", bufs=1))
        v_t = v_p.tile([P, NS, E], bf16)
        vs_p = ctx.enter_context(tc.tile_pool(name="vs", bufs=1))
        vHi = vs_p.tile([P, NS, E], fp8)
        vLo = vs_p.tile([P, NS, E], fp8)
        y_p = ctx.enter_context(tc.tile_pool(name="y", bufs=1))
        yT8 = y_p.tile([P, NE, S], fp8)

        # one PSUM pool for the whole kernel: 4 x [P,1024] f32 (2 zero
        # regions each; chains stay within one 512-col region)
        ps = ctx.enter_context(tc.tile_pool(name="ps", bufs=4, space="PSUM"))

        with ExitStack() as ph1:
            w_p = ph1.enter_context(tc.tile_pool(name="w", bufs=1))
            wvh_t = w_p.tile([P, NE, E], fp8, tag="wvh")
            wvl_t = w_p.tile([P, NE, E], fp8, tag="wvl")
            ahi_t = w_p.tile([P, NE, E], fp8, tag="ahi")
            alo_t = w_p.tile([P, NE, E], fp8, tag="alo")
            uw_t = w_p.tile([P, NE, 16], fp8, tag="uw")

            # stage DMAs so v-chains can start as soon as possible:
            # interleave xhi/wvh e-pairs, then wvl, then xlo, then A
            for ep in range(4):
                e2 = slice(2 * ep, 2 * ep + 2)
                nc.sync.dma_start(xhi[:, e2, :], xhi_d[:, e2, :])
                nc.sync.dma_start(wvh_t[:, e2, :], wvh_d[:, e2, :])
            nc.sync.dma_start(wvl_t[:], wvl_d)
            nc.sync.dma_start(xlo[:], xlo_d)
            nc.sync.dma_start(uw_t[:], uw_d)
            nc.sync.dma_start(cc_t[:], cc_d)
            nc.sync.dma_start(bv_t[:], bv_d)
            nc.sync.dma_start(ahi_t[:], ahi_d)
            nc.sync.dma_start(alo_t[:], alo_d)

            # ---- v32 = x@(32Wv) + 32bv, 3-split chains; static v-split --
            for j in range(NS):
                js = slice(j * P, (j + 1) * P)
                pv = ps.tile([P, 1024], f32, tag="ps")
                for h in range(2):
                    hs = slice(h * 512, (h + 1) * 512)
                    for ep in range(4):
                        e2 = slice(2 * ep, 2 * ep + 2)
                        nc.tensor.matmul(pv[:, hs], xhi[:, e2, js],
                                         wvh_t[:, e2, hs],
                                         start=(ep == 0), stop=False,
                                         perf_mode=DR)
                    for ep in range(4):
                        e2 = slice(2 * ep, 2 * ep + 2)
                        nc.tensor.matmul(pv[:, hs], xhi[:, e2, js],
                                         wvl_t[:, e2, hs],
                                         start=False, stop=False,
                                         perf_mode=DR)
                    for ep in range(4):
                        e2 = slice(2 * ep, 2 * ep + 2)
                        nc.tensor.matmul(pv[:, hs], xlo[:, e2, js],
                                         wvh_t[:, e2, hs],
                                         start=False, stop=(ep == 3),
                                         perf_mode=DR)
                    nc.vector.tensor_tensor(v_t[:, j, hs], pv[:, hs],
                                            bv_t[:, hs], op=Alu.add)
                # static fp8 split of v32 (no Z dependency)
                nc.gpsimd.tensor_copy(vHi[:, j, :], v_t[:, j, :])
                nc.vector.tensor_tensor(vLo[:, j, :], v_t[:, j, :],
                                        vHi[:, j, :], op=Alu.subtract)

            # ---- rank-1 rows: rr[0,:] = 64 r1, rr[1,:] = 64 r2 ----
            for cq2 in range(2):
                pr = ps.tile([16, 1024], f32, tag="ps")
                for q in range(2):
                    qs = slice(q * 512, (q + 1) * 512)
                    cs = slice((2 * cq2 + q) * 512, (2 * cq2 + q + 1) * 512)
                    for ep in range(4):
                        e2 = slice(2 * ep, 2 * ep + 2)
                        nc.tensor.matmul(pr[:, qs], uw_t[:, e2, :],
                                         xhi[:, e2, cs],
                                         start=(ep == 0), stop=(ep == 3),
                                         perf_mode=DR)
                cs2 = slice(2 * cq2 * 512, (2 * cq2 + 2) * 512)
                nc.vector.tensor_copy(rr_t[:, cs2], pr[0:2, :])
            # r2 -> [P, NS] via DRAM round trip; bias = (64r2 + 64c)/2048
            nc.sync.dma_start(r2_d[:, :], rr_t[1:2, :])
            nc.sync.dma_start(
                r2T[:], r2_d[0:1, :].rearrange("a (t p) -> (a p) t", p=P))
            nc.vector.tensor_scalar(bias_t[:], r2T[:], cc_t[:, 0:1], SC,
                                    op0=Alu.add, op1=Alu.mult)
            # r1 row -> fp8, broadcast into both DoubleRow k-pair slots
            nc.vector.tensor_copy(r1r8[:], rr_t[0:1, :])
            nc.gpsimd.partition_broadcast(r1b[:, 0, :], r1r8[:])
            nc.gpsimd.partition_broadcast(r1b[:, 1, :], r1r8[:])

            # ---- yT8[d, i] = fp8(64 (x@A)^T), 3-split chains ----
            for d in range(NE):
                ds = slice(d * P, (d + 1) * P)
                for cq2 in range(2):
                    pq = ps.tile([P, 1024], f32, tag="ps")
                    for q in range(2):
                        qs = slice(q * 512, (q + 1) * 512)
                        cs = slice((2 * cq2 + q) * 512, (2 * cq2 + q + 1) * 512)
                        for ep in range(4):
                            e2 = slice(2 * ep, 2 * ep + 2)
                            nc.tensor.matmul(pq[:, qs], ahi_t[:, e2, ds],
                                             xhi[:, e2, cs],
                                             start=(ep == 0), stop=False,
                                             perf_mode=DR)
                        for ep in range(4):
                            e2 = slice(2 * ep, 2 * ep + 2)
                            nc.tensor.matmul(pq[:, qs], alo_t[:, e2, ds],
                                             xhi[:, e2, cs],
                                             start=False, stop=False,
                                             perf_mode=DR)
                        for ep in range(4):
                            e2 = slice(2 * ep, 2 * ep + 2)
                            nc.tensor.matmul(pq[:, qs], ahi_t[:, e2, ds],
                                             xlo[:, e2, cs],
                                             start=False, stop=(ep == 3),
                                             perf_mode=DR)
                    cs2 = slice(2 * cq2 * 512, (2 * cq2 + 2) * 512)
                    nc.scalar.copy(yT8[:, d, cs2], pq[:])

        # ---- scoresT + exp(+Z accum) + z-folded Ep8 cast, per j-tile ----
        ep_p = ctx.enter_context(tc.tile_pool(name="ep", bufs=1))
        Ep8 = ep_p.tile([P, NS, S], fp8)
        et_p = ctx.enter_context(tc.tile_pool(name="et", bufs=3))

        for j in range(NS):
            js = slice(j * P, (j + 1) * P)
            et = et_p.tile([P, S], bf16, tag="et")
            for h in range(2):
                pt = ps.tile([P, 1024], f32, tag="ps")
                for q in range(2):
                    qs = slice(q * 512, (q + 1) * 512)
                    gcs = slice(h * 1024 + q * 512, h * 1024 + (q + 1) * 512)
                    for dp in range(4):
                        d2 = slice(2 * dp, 2 * dp + 2)
                        nc.tensor.matmul(pt[:, qs], xhi[:, d2, js],
                                         yT8[:, d2, gcs],
                                         start=(dp == 0), stop=False,
                                         perf_mode=DR)
                    nc.tensor.matmul(pt[:, qs], c0[:], r1b[:, :, gcs],
                                     start=False, stop=True, perf_mode=DR)
                nc.scalar.activation(et[:, h * 1024:(h + 1) * 1024], pt[:],
                                     func=Act.Exp, scale=SC,
                                     bias=bias_t[:, j:j + 1],
                                     accum_out=zp[:, 2 * j + h:2 * j + h + 1])
            # sA_j = 8192/Z_j   (Z_j = sum of both halves' accums)
            nc.vector.tensor_tensor(ztm[:, j:j + 1], zp[:, 2 * j:2 * j + 1],
                                    zp[:, 2 * j + 1:2 * j + 2], op=Alu.add)
            nc.vector.tensor_scalar_mul(ztm[:, j:j + 1], ztm[:, j:j + 1],
                                        1.0 / 8192.0)
            nc.vector.reciprocal(sA[:, j:j + 1], ztm[:, j:j + 1])
            # Ep8 = fp8((E - 1) * sA_j), halves split across DVE/Pool
            nc.vector.tensor_scalar(Ep8[:, j, 0:1024], et[:, 0:1024],
                                    1.0, sA[:, j:j + 1],
                                    op0=Alu.subtract, op1=Alu.mult)
            nc.gpsimd.tensor_scalar(Ep8[:, j, 1024:2048], et[:, 1024:2048],
                                    1.0, sA[:, j:j + 1],
                                    op0=Alu.subtract, op1=Alu.mult)

        # ---- out phase ----
        ob_p = ctx.enter_context(tc.tile_pool(name="ob", bufs=3))

        # z-derived scalars for the mean rows
        nc.vector.tensor_reduce(zrow[:], zp[:], axis=AxX, op=Alu.add)
        nc.gpsimd.tensor_reduce(zscal[:], zrow[:], axis=AxC, op=Alu.add)
        nc.vector.reciprocal(zbi[:], zscal[:])                 # 1/(2048 Zbar)
        nc.vector.tensor_scalar_mul(zq[:], zscal[:], 1.0 / 262144.0)
        nc.gpsimd.partition_broadcast(zbp[:], zq[:])           # Zbar/128
        nc.vector.scalar_tensor_tensor(dzf[:], sA[:], zbp[:, 0:1], c64[:],
                                       op0=Alu.mult, op1=Alu.subtract)
        for k2 in range(16):
            nc.vector.tensor_copy(dz8rep[:, :, k2], dzf[:])

        # B rows: B1 (exact const 4) and B2 (fp8 dz residual)
        pb1s, pb2s = [], []
        for h in range(2):
            hs = slice(h * 512, (h + 1) * 512)
            pb = ps.tile([16, 1024], f32, tag="ps")
            for jp in range(8):
                j2 = slice(2 * jp, 2 * jp + 2)
                nc.tensor.matmul(pb[:, 0:512], c4[:], vHi[:, j2, hs],
                                 start=(jp == 0), stop=False, perf_mode=DR)
            for jp in range(8):
                j2 = slice(2 * jp, 2 * jp + 2)
                nc.tensor.matmul(pb[:, 0:512], c4[:], vLo[:, j2, hs],
                                 start=False, stop=(jp == 7), perf_mode=DR)
            for jp in range(8):
                j2 = slice(2 * jp, 2 * jp + 2)
                nc.tensor.matmul(pb[:, 512:1024], dz8rep[:, j2, :],
                                 vHi[:, j2, hs],
                                 start=(jp == 0), stop=False, perf_mode=DR)
            for jp in range(8):
                j2 = slice(2 * jp, 2 * jp + 2)
                nc.tensor.matmul(pb[:, 512:1024], dz8rep[:, j2, :],
                                 vLo[:, j2, hs],
                                 start=False, stop=(jp == 7), perf_mode=DR)
            nc.scalar.activation(rowS[0:1, hs], pb[0:1, 0:512],
                                 func=Act.Copy, scale=16.0)
            nc.vector.scalar_tensor_tensor(rowT[0:1, hs], pb[0:1, 512:1024],
                                           1.0, rowS[0:1, hs],
                                           op0=Alu.mult, op1=Alu.add)
        nc.scalar.activation(rowF[:], rowT[:], func=Act.Copy,
                             scale=zbi[0:1, 0:1])
        nc.gpsimd.partition_broadcast(mbc[:], rowF[:])

        # A chains: out[i,:] = psumA/262144 + mbc
        for i in range(NS):
            isl = slice(i * P, (i + 1) * P)
            ob = ob_p.tile([P, E], f32, tag="ob")
            po = ps.tile([P, 1024], f32, tag="ps")
            for h in range(2):
                hs = slice(h * 512, (h + 1) * 512)
                for jp in range(8):
                    j2 = slice(2 * jp, 2 * jp + 2)
                    nc.tensor.matmul(po[:, hs], Ep8[:, j2, isl],
                                     vHi[:, j2, hs],
                                     start=(jp == 0), stop=False,
                                     perf_mode=DR)
                for jp in range(8):
                    j2 = slice(2 * jp, 2 * jp + 2)
                    nc.tensor.matmul(po[:, hs], Ep8[:, j2, isl],
                                     vLo[:, j2, hs],
                                     start=False, stop=(jp == 7),
                                     perf_mode=DR)
                nc.vector.scalar_tensor_tensor(ob[:, hs], po[:, hs],
                                               1.0 / 262144.0, mbc[:, hs],
                                               op0=Alu.mult, op1=Alu.add)
            nc.sync.dma_start(out_d[isl, 0:512], ob[:, 0:512])
            nc.sync.dma_start(out_d[isl, 512:1024], ob[:, 512:1024])


def _get_built():
    if "nc" not in _BUILT:
        _BUILT["nc"] = _build()
    return _BUILT["nc"]


F8 = ml_dtypes.float8_e4m3fn


def _tile_w(w):
    # [E, E] f32 -> PE tile layout [P, NE, E]: [p, e, d] = W[e*P + p, d]
    return np.ascontiguousarray(
        np.asarray(w, dtype=np.float32).reshape(NE, P, E).transpose(1, 0, 2))


def _split8(a32):
    hi = a32.astype(F8)
    lo = (a32 - hi.astype(np.float32)).astype(F8)
    return hi, lo


def _make_in_maps(inputs):
    x = np.asarray(inputs["x_h"], dtype=np.float32)     # [8, S, E]
    Wq = np.asarray(inputs["Wq"], dtype=np.float64)
    bq = np.asarray(inputs["bq"], dtype=np.float64)
    Wk = np.asarray(inputs["Wk"], dtype=np.float64)
    bk = np.asarray(inputs["bk"], dtype=np.float64)
    Wv = np.asarray(inputs["Wv"], dtype=np.float64)
    bv = np.asarray(inputs["bv"], dtype=np.float64)

    # host weight folding (fp64)
    A = Wq @ Wk.T                                       # [E, E]
    u = Wq @ bk                                         # [E]
    w = Wk @ bq                                         # [E]
    c = float(bq @ bk)

    ahi_h, alo_h = _split8(_tile_w(64.0 * A))
    wvh_h, wvl_h = _split8(_tile_w(32.0 * Wv))
    uw_h = np.zeros((P, NE, 16), dtype=np.float32)      # [P, NE, 16] padded
    uw_h[:, :, 0] = (64.0 * u).astype(np.float32).reshape(NE, P).T
    uw_h[:, :, 1] = (64.0 * w).astype(np.float32).reshape(NE, P).T
    uw_h = uw_h.astype(F8)
    cc_h = np.full((P, 1), 64.0 * c, dtype=np.float32)
    bv_h = np.ascontiguousarray(
        np.broadcast_to((32.0 * bv).astype(np.float32).reshape(1, E),
                        (P, E))).astype(ml_dtypes.bfloat16)

    in_maps = []
    for b in range(NCORES):
        # xT tile layout [P, NE, S]: [p, e, i] = x[b][i, e*P + p]
        xT = np.ascontiguousarray(
            x[b].T.reshape(NE, P, S).transpose(1, 0, 2))
        xhi_h, xlo_h = _split8(xT)
        in_maps.append({
            "xhi": xhi_h, "xlo": xlo_h, "ahi": ahi_h, "alo": alo_h,
            "wvh": wvh_h, "wvl": wvl_h, "uw": uw_h,
            "bv": bv_h, "cc": cc_h,
        })
    return in_maps


def kernel(**inputs):
    from concourse.bass_utils import run_bass_kernel_spmd

    nc = _get_built()
    in_maps = _make_in_maps(inputs)
    res = run_bass_kernel_spmd(nc, in_maps, list(range(NCORES)))
    out = np.stack([np.asarray(res.results[b]["out"], dtype=np.float32)
                    for b in range(NCORES)])
    return out


# revision 11
# speedup vs baseline: 1.6034x; 1.0005x over previous
"""Trainium2 Bass kernel for nn_AttentionBlock (B=8, S=2048, D=1024).

Reference computation (per batch element b):
    q = x @ Wq + bq ; k = x @ Wk + bk ; v = x @ Wv + bv
    scores = (q @ k^T) / sqrt(1024)
    attn = softmax(scores, axis=QUERY)          # axis=1 of [B, S_q, S_k]!
    out = attn @ v

Sharding: pure data-parallel — batch element b runs on NeuronCore b.

Device algorithm — fp8e4m3 matmuls in DoubleRow perf mode (two 128-deep
k-tiles per instruction at 0.5 cycles/output-column = 4x the bf16 MAC
rate), fp32 PSUM accumulation, out-free 512 per matmul.  Precision is
held inside the rel-err budget by hi/lo operand splitting
(a ~ fp8(a) + fp8(a - fp8(a))) on the paths where quantization error
passes straight through to the output:

  - weight folding (host, fp64): A = Wq Wk^T, u = Wq bk, w = Wk bq,
    c = bq.bk, so scores_raw[i,j] = x_i A x_j^T + r1_i + r2_j + c with
    r1 = x u, r2 = x w.  Removes the separate q/k projections.
  - host supplies xT in fp8 hi+lo ([P, e, i] PE tile layout), A*64 and
    Wv*32 in fp8 hi+lo (scaled to dodge fp8 subnormals; the scales
    fold into the exp argument / output copy).
  - v32 = x@(32Wv)+32bv: 3-term split -> bf16, then a STATIC fp8 hi/lo
    split (vHi/vLo) during phase 1 — no Z dependency.
  - y = x@(64A): 3-term split -> yT8 = fp8(64y).
  - scoresT[j,i] = xHi[j].yT8[i] + 64*r1_i; the rank-1 r1 row rides in
    as one extra DoubleRow matmul per chain (const 1/256 stationary x
    broadcast fp8 r1 row), so exp() emits the FULL softmax numerator
    E[j,i] and the activation accumulator produces Z_j for free.
  - The 1/Z_j softmax fold happens on the E side (keys = partitions of
    the Et tile): one DVE/Pool tensor_scalar pass casts
    Ep8 = fp8((E - 1) * 8192/Z_j), which simultaneously mean-centers E
    (3x smaller fp8 quantization error — no E-lo chain needed).
  - The dropped softmax mean sum_j v[j,:]/Z_j is restored exactly by
    two [16,512] psum row-chains over vHi/vLo: B1 with an exact
    const-4 stationary (carries the 1/Zbar part) and B2 with a small
    fp8 residual stationary dz = 64*(Zbar/Z_j - 1) (±2% values, so its
    quantization is second-order).  Their combination is broadcast and
    added during the final PSUM->SBUF copies.
  - out[i,:] = psumA/262144 + (16*B1 + B2)/(2048*Zbar).
"""

import numpy as np
import ml_dtypes

S = 2048          # sequence length
E = 1024          # emb dim == att dim
P = 128           # partitions
NS = S // P       # 16 sequence tiles
NE = E // P       # 8 emb tiles
NCORES = 8
SC = 1.0 / 2048.0  # exp scale on the x64-scaled psum: (1/32)*(1/64)

_BUILT = {}


def _build(reps=1):
    """Construct the Bass program (same NEFF for all 8 cores)."""
    import concourse.tile as tile
    import concourse.mybir as mybir
    from concourse import bacc

    nc = bacc.Bacc("TRN2", target_bir_lowering=False, debug=False)

    f32 = mybir.dt.float32
    bf16 = mybir.dt.bfloat16
    fp8 = mybir.dt.float8e4

    xhi_d = nc.dram_tensor("xhi", [P, NE, S], fp8, kind="ExternalInput").ap()
    xlo_d = nc.dram_tensor("xlo", [P, NE, S], fp8, kind="ExternalInput").ap()
    ahi_d = nc.dram_tensor("ahi", [P, NE, E], fp8, kind="ExternalInput").ap()
    alo_d = nc.dram_tensor("alo", [P, NE, E], fp8, kind="ExternalInput").ap()
    wvh_d = nc.dram_tensor("wvh", [P, NE, E], fp8, kind="ExternalInput").ap()
    wvl_d = nc.dram_tensor("wvl", [P, NE, E], fp8, kind="ExternalInput").ap()
    uw_d = nc.dram_tensor("uw", [P, NE, 16], fp8, kind="ExternalInput").ap()
    bv_d = nc.dram_tensor("bv", [P, E], bf16, kind="ExternalInput").ap()
    cc_d = nc.dram_tensor("cc", [P, 1], f32, kind="ExternalInput").ap()
    out_d = nc.dram_tensor("out", [S, E], f32, kind="ExternalOutput").ap()
    r2_d = nc.dram_tensor("r2scratch", [1, S], f32).ap()  # internal

    with tile.TileContext(nc) as tc:
        for _ in range(reps):
            _emit_body(nc, tc, xhi_d, xlo_d, ahi_d, alo_d, wvh_d, wvl_d,
                       uw_d, bv_d, cc_d, out_d, r2_d)

    nc.compile()
    return nc


def _emit_body(nc, tc, xhi_d, xlo_d, ahi_d, alo_d, wvh_d, wvl_d,
               uw_d, bv_d, cc_d, out_d, r2_d):
    from contextlib import ExitStack
    import concourse.mybir as mybir

    f32 = mybir.dt.float32
    bf16 = mybir.dt.bfloat16
    fp8 = mybir.dt.float8e4
    Act = mybir.ActivationFunctionType
    Alu = mybir.AluOpType
    DR = mybir.MatmulPerfMode.DoubleRow
    from concourse import bass_isa
    AxX = mybir.AxisListType.X
    RedOp = bass_isa.ReduceOp

    with ExitStack() as ctx:
        const_p = ctx.enter_context(tc.tile_pool(name="const", bufs=1))
        bv_t = const_p.tile([P, E], bf16)
        cc_t = const_p.tile([P, 1], f32)
        rr_t = const_p.tile([2, S], f32)
        r1b = const_p.tile([P, 2, S], fp8)
        r1r8 = const_p.tile([1, S], fp8)
        r2T = const_p.tile([P, NS], f32)
        bias_t = const_p.tile([P, NS], f32)
        zp = const_p.tile([P, 2 * NS], f32)   # per-(j,half) exp accums
        ztm = const_p.tile([P, NS], f32)
        sA = const_p.tile([P, NS], f32)       # 8192 / Z_j
        c0 = const_p.tile([P, 2, P], fp8)     # 1/256 stationary (r1 add)
        c4 = const_p.tile([P, 2, 16], fp8)    # 4.0 stationary (B1 row)
        c64 = const_p.tile([P, NS], f32)      # 64.0
        dzf = const_p.tile([P, NS], f32)
        dz8rep = const_p.tile([P, NS, 16], fp8)
        zrow = const_p.tile([P, 1], f32)
        zall = const_p.tile([P, 1], f32)      # 2048 * Zbar (all parts)
        zbi = const_p.tile([1, 1], f32)       # 1/(2048 Zbar)
        zbp = const_p.tile([P, 1], f32)       # Zbar/128
        rowT = const_p.tile([1, E], f32)
        rowS = const_p.tile([1, E], f32)
        rowF = const_p.tile([1, E], f32)
        mbc = const_p.tile([P, E], f32)
        nc.vector.memset(c0[:], 1.0 / 256.0)
        nc.vector.memset(c4[:], 4.0)
        nc.vector.memset(c64[:], 64.0)

        x_p = ctx.enter_context(tc.tile_pool(name="x", bufs=1))
        xhi = x_p.tile([P, NE, S], fp8)
        xlo = x_p.tile([P, NE, S], fp8)
        v_p = ctx.enter_context(tc.tile_pool(name="v", bufs=1))
        v_t = v_p.tile([P, NS, E], bf16)
        vs_p = ctx.enter_context(tc.tile_pool(name="vs", bufs=1))
        vHi = vs_p.tile([P, NS, E], fp8)
        vLo = vs_p.tile([P, NS, E], fp8)
        y_p = ctx.enter_context(tc.tile_pool(name="y", bufs=1))
        yT8 = y_p.tile([P, NE, S], fp8)

        # one PSUM pool for the whole kernel: 4 x [P,1024] f32 (2 zero
        # regions each; chains stay within one 512-col region)
        ps = ctx.enter_context(tc.tile_pool(name="ps", bufs=4, space="PSUM"))

        with ExitStack() as ph1:
            w_p = ph1.enter_context(tc.tile_pool(name="w", bufs=1))
            wvh_t = w_p.tile([P, NE, E], fp8, tag="wvh")
            wvl_t = w_p.tile([P, NE, E], fp8, tag="wvl")
            ahi_t = w_p.tile([P, NE, E], fp8, tag="ahi")
            alo_t = w_p.tile([P, NE, E], fp8, tag="alo")
            uw_t = w_p.tile([P, NE, 16], fp8, tag="uw")

            # stage DMAs so v-chains can start as soon as possible:
            # interleave xhi/wvh e-pairs, then wvl, then xlo, then A
            # spread input DMAs across four engine queues so the early
            # phases aren't gated on one DGE ring
            for ep in range(4):
                e2 = slice(2 * ep, 2 * ep + 2)
                nc.sync.dma_start(xhi[:, e2, :], xhi_d[:, e2, :])
                nc.scalar.dma_start(wvh_t[:, e2, :], wvh_d[:, e2, :])
                nc.gpsimd.dma_start(wvl_t[:, e2, :], wvl_d[:, e2, :])
            for ep in range(2):
                e2 = slice(2 * ep, 2 * ep + 2)
                nc.scalar.dma_start(xlo[:, e2, :], xlo_d[:, e2, :])
            for ep in range(2, 4):
                e2 = slice(2 * ep, 2 * ep + 2)
                nc.gpsimd.dma_start(xlo[:, e2, :], xlo_d[:, e2, :])
            nc.sync.dma_start(uw_t[:], uw_d)
            nc.sync.dma_start(cc_t[:], cc_d)
            nc.sync.dma_start(bv_t[:], bv_d)
            nc.sync.dma_start(ahi_t[:], ahi_d)
            nc.gpsimd.dma_start(alo_t[:], alo_d)

            # ---- v32 = x@(32Wv) + 32bv, 3-split chains; static v-split --
            for j in range(NS):
                js = slice(j * P, (j + 1) * P)
                pv = ps.tile([P, 1024], f32, tag="ps")
                for h in range(2):
                    hs = slice(h * 512, (h + 1) * 512)
                    for ep in range(4):
                        e2 = slice(2 * ep, 2 * ep + 2)
                        nc.tensor.matmul(pv[:, hs], xhi[:, e2, js],
                                         wvh_t[:, e2, hs],
                                         start=(ep == 0), stop=False,
                                         perf_mode=DR)
                    for ep in range(4):
                        e2 = slice(2 * ep, 2 * ep + 2)
                        nc.tensor.matmul(pv[:, hs], xhi[:, e2, js],
                                         wvl_t[:, e2, hs],
                                         start=False, stop=False,
                                         perf_mode=DR)
                    for ep in range(4):
                        e2 = slice(2 * ep, 2 * ep + 2)
                        nc.tensor.matmul(pv[:, hs], xlo[:, e2, js],
                                         wvh_t[:, e2, hs],
                                         start=False, stop=(ep == 3),
                                         perf_mode=DR)
                    nc.vector.tensor_tensor(v_t[:, j, hs], pv[:, hs],
                                            bv_t[:, hs], op=Alu.add)
                # static fp8 split of v32 (no Z dependency)
                nc.gpsimd.tensor_copy(vHi[:, j, :], v_t[:, j, :])
                nc.vector.tensor_tensor(vLo[:, j, :], v_t[:, j, :],
                                        vHi[:, j, :], op=Alu.subtract)

            # ---- rank-1 rows: rr[0,:] = 64 r1, rr[1,:] = 64 r2 ----
            for cq2 in range(2):
                pr = ps.tile([16, 1024], f32, tag="ps")
                for q in range(2):
                    qs = slice(q * 512, (q + 1) * 512)
                    cs = slice((2 * cq2 + q) * 512, (2 * cq2 + q + 1) * 512)
                    for ep in range(4):
                        e2 = slice(2 * ep, 2 * ep + 2)
                        nc.tensor.matmul(pr[:, qs], uw_t[:, e2, :],
                                         xhi[:, e2, cs],
                                         start=(ep == 0), stop=(ep == 3),
                                         perf_mode=DR)
                cs2 = slice(2 * cq2 * 512, (2 * cq2 + 2) * 512)
                nc.vector.tensor_copy(rr_t[:, cs2], pr[0:2, :])
            # r2 -> [P, NS] via DRAM round trip; bias = (64r2 + 64c)/2048
            nc.sync.dma_start(r2_d[:, :], rr_t[1:2, :])
            nc.sync.dma_start(
                r2T[:], r2_d[0:1, :].rearrange("a (t p) -> (a p) t", p=P))
            nc.vector.tensor_scalar(bias_t[:], r2T[:], cc_t[:, 0:1], SC,
                                    op0=Alu.add, op1=Alu.mult)
            # r1 row -> fp8, broadcast into both DoubleRow k-pair slots
            nc.vector.tensor_copy(r1r8[:], rr_t[0:1, :])
            nc.gpsimd.partition_broadcast(r1b[:, 0, :], r1r8[:])
            nc.gpsimd.partition_broadcast(r1b[:, 1, :], r1r8[:])

            # ---- yT8[d, i] = fp8(64 (x@A)^T), 3-split chains ----
            for d in range(NE):
                ds = slice(d * P, (d + 1) * P)
                for cq2 in range(2):
                    pq = ps.tile([P, 1024], f32, tag="ps")
                    for q in range(2):
                        qs = slice(q * 512, (q + 1) * 512)
                        cs = slice((2 * cq2 + q) * 512, (2 * cq2 + q + 1) * 512)
                        for ep in range(4):
                            e2 = slice(2 * ep, 2 * ep + 2)
                            nc.tensor.matmul(pq[:, qs], ahi_t[:, e2, ds],
                                             xhi[:, e2, cs],
                                             start=(ep == 0), stop=False,
                                             perf_mode=DR)
                        for ep in range(4):
                            e2 = slice(2 * ep, 2 * ep + 2)
                            nc.tensor.matmul(pq[:, qs], alo_t[:, e2, ds],
                                             xhi[:, e2, cs],
                                             start=False, stop=False,
                                             perf_mode=DR)
                        for ep in range(4):
                            e2 = slice(2 * ep, 2 * ep + 2)
                            nc.tensor.matmul(pq[:, qs], ahi_t[:, e2, ds],
                                             xlo[:, e2, cs],
                                             start=False, stop=(ep == 3),
                                             perf_mode=DR)
                    cs2 = slice(2 * cq2 * 512, (2 * cq2 + 2) * 512)
                    nc.scalar.copy(yT8[:, d, cs2], pq[:])

        # ---- scoresT + exp(+Z accum) + z-folded Ep8 cast, per j-tile ----
        ep_p = ctx.enter_context(tc.tile_pool(name="ep", bufs=1))
        Ep8 = ep_p.tile([P, NS, S], fp8)
        et_p = ctx.enter_context(tc.tile_pool(name="et", bufs=4))

        for j in range(NS):
            js = slice(j * P, (j + 1) * P)
            et = et_p.tile([P, S], bf16, tag="et")
            for h in range(2):
                pt = ps.tile([P, 1024], f32, tag="ps")
                for q in range(2):
                    qs = slice(q * 512, (q + 1) * 512)
                    gcs = slice(h * 1024 + q * 512, h * 1024 + (q + 1) * 512)
                    for dp in range(4):
                        d2 = slice(2 * dp, 2 * dp + 2)
                        nc.tensor.matmul(pt[:, qs], xhi[:, d2, js],
                                         yT8[:, d2, gcs],
                                         start=(dp == 0), stop=False,
                                         perf_mode=DR)
                    nc.tensor.matmul(pt[:, qs], c0[:], r1b[:, :, gcs],
                                     start=False, stop=True, perf_mode=DR)
                nc.scalar.activation(et[:, h * 1024:(h + 1) * 1024], pt[:],
                                     func=Act.Exp, scale=SC,
                                     bias=bias_t[:, j:j + 1],
                                     accum_out=zp[:, 2 * j + h:2 * j + h + 1])
            # sA_j = 8192/Z_j   (Z_j = sum of both halves' accums)
            nc.vector.tensor_tensor(ztm[:, j:j + 1], zp[:, 2 * j:2 * j + 1],
                                    zp[:, 2 * j + 1:2 * j + 2], op=Alu.add)
            nc.vector.tensor_scalar_mul(ztm[:, j:j + 1], ztm[:, j:j + 1],
                                        1.0 / 8192.0)
            nc.vector.reciprocal(sA[:, j:j + 1], ztm[:, j:j + 1])
            # Ep8 = fp8((E - 1) * sA_j), halves split across DVE/Pool
            nc.vector.tensor_scalar(Ep8[:, j, 0:1024], et[:, 0:1024],
                                    1.0, sA[:, j:j + 1],
                                    op0=Alu.subtract, op1=Alu.mult)
            nc.gpsimd.tensor_scalar(Ep8[:, j, 1024:2048], et[:, 1024:2048],
                                    1.0, sA[:, j:j + 1],
                                    op0=Alu.subtract, op1=Alu.mult)

        # ---- out phase ----
        ob_p = ctx.enter_context(tc.tile_pool(name="ob", bufs=2))

        # z-derived scalars for the mean rows
        nc.vector.tensor_reduce(zrow[:], zp[:], axis=AxX, op=Alu.add)
        nc.gpsimd.partition_all_reduce(zall[:], zrow[:], channels=P,
                                       reduce_op=RedOp.add)    # 2048 Zbar
        nc.vector.reciprocal(zbi[:], zall[0:1, 0:1])           # 1/(2048 Zbar)
        nc.vector.tensor_scalar_mul(zbp[:], zall[:], 1.0 / 262144.0)
        nc.vector.scalar_tensor_tensor(dzf[:], sA[:], zbp[:, 0:1], c64[:],
                                       op0=Alu.mult, op1=Alu.subtract)
        for k2 in range(16):
            nc.vector.tensor_copy(dz8rep[:, :, k2], dzf[:])

        # B rows: B1 (exact const 4) and B2 (fp8 dz residual)
        pb1s, pb2s = [], []
        for h in range(2):
            hs = slice(h * 512, (h + 1) * 512)
            pb = ps.tile([16, 1024], f32, tag="ps")
            for jp in range(8):
                j2 = slice(2 * jp, 2 * jp + 2)
                nc.tensor.matmul(pb[:, 0:512], c4[:], vHi[:, j2, hs],
                                 start=(jp == 0), stop=False, perf_mode=DR)
            for jp in range(8):
                j2 = slice(2 * jp, 2 * jp + 2)
                nc.tensor.matmul(pb[:, 0:512], c4[:], vLo[:, j2, hs],
                                 start=False, stop=(jp == 7), perf_mode=DR)
            for jp in range(8):
                j2 = slice(2 * jp, 2 * jp + 2)
                nc.tensor.matmul(pb[:, 512:1024], dz8rep[:, j2, :],
                                 vHi[:, j2, hs],
                                 start=(jp == 0), stop=False, perf_mode=DR)
            for jp in range(8):
                j2 = slice(2 * jp, 2 * jp + 2)
                nc.tensor.matmul(pb[:, 512:1024], dz8rep[:, j2, :],
                                 vLo[:, j2, hs],
                                 start=False, stop=(jp == 7), perf_mode=DR)
            nc.scalar.activation(rowS[0:1, hs], pb[0:1, 0:512],
                                 func=Act.Copy, scale=16.0)
            nc.vector.scalar_tensor_tensor(rowT[0:1, hs], pb[0:1, 512:1024],
                                           1.0, rowS[0:1, hs],
                                           op0=Alu.mult, op1=Alu.add)
        nc.scalar.activation(rowF[:], rowT[:], func=Act.Copy,
                             scale=zbi[0:1, 0:1])
        nc.gpsimd.partition_broadcast(mbc[:], rowF[:])

        # A chains: out[i,:] = psumA/262144 + mbc
        for i in range(NS):
            isl = slice(i * P, (i + 1) * P)
            ob = ob_p.tile([P, E], f32, tag="ob")
            po = ps.tile([P, 1024], f32, tag="ps")
            for h in range(2):
                hs = slice(h * 512, (h + 1) * 512)
                for jp in range(8):
                    j2 = slice(2 * jp, 2 * jp + 2)
                    nc.tensor.matmul(po[:, hs], Ep8[:, j2, isl],
                                     vHi[:, j2, hs],
                                     start=(jp == 0), stop=False,
                                     perf_mode=DR)
                for jp in range(8):
                    j2 = slice(2 * jp, 2 * jp + 2)
                    nc.tensor.matmul(po[:, hs], Ep8[:, j2, isl],
                                     vLo[:, j2, hs],
                                     start=False, stop=(jp == 7),
                                     perf_mode=DR)
                nc.vector.scalar_tensor_tensor(ob[:, hs], po[:, hs],
                                               1.0 / 262144.0, mbc[:, hs],
                                               op0=Alu.mult, op1=Alu.add)
            nc.sync.dma_start(out_d[isl, 0:512], ob[:, 0:512])
            nc.sync.dma_start(out_d[isl, 512:1024], ob[:, 512:1024])


def _get_built():
    if "nc" not in _BUILT:
        _BUILT["nc"] = _build()
    return _BUILT["nc"]


F8 = ml_dtypes.float8_e4m3fn


def _tile_w(w):
    # [E, E] f32 -> PE tile layout [P, NE, E]: [p, e, d] = W[e*P + p, d]
    return np.ascontiguousarray(
        np.asarray(w, dtype=np.float32).reshape(NE, P, E).transpose(1, 0, 2))


def _split8(a32):
    hi = a32.astype(F8)
    lo = (a32 - hi.astype(np.float32)).astype(F8)
    return hi, lo


def _make_in_maps(inputs):
    x = np.asarray(inputs["x_h"], dtype=np.float32)     # [8, S, E]
    Wq = np.asarray(inputs["Wq"], dtype=np.float64)
    bq = np.asarray(inputs["bq"], dtype=np.float64)
    Wk = np.asarray(inputs["Wk"], dtype=np.float64)
    bk = np.asarray(inputs["bk"], dtype=np.float64)
    Wv = np.asarray(inputs["Wv"], dtype=np.float64)
    bv = np.asarray(inputs["bv"], dtype=np.float64)

    # host weight folding (fp64)
    A = Wq @ Wk.T                                       # [E, E]
    u = Wq @ bk                                         # [E]
    w = Wk @ bq                                         # [E]
    c = float(bq @ bk)

    ahi_h, alo_h = _split8(_tile_w(64.0 * A))
    wvh_h, wvl_h = _split8(_tile_w(32.0 * Wv))
    uw_h = np.zeros((P, NE, 16), dtype=np.float32)      # [P, NE, 16] padded
    uw_h[:, :, 0] = (64.0 * u).astype(np.float32).reshape(NE, P).T
    uw_h[:, :, 1] = (64.0 * w).astype(np.float32).reshape(NE, P).T
    uw_h = uw_h.astype(F8)
    cc_h = np.full((P, 1), 64.0 * c, dtype=np.float32)
    bv_h = np.ascontiguousarray(
        np.broadcast_to((32.0 * bv).astype(np.float32).reshape(1, E),
                        (P, E))).astype(ml_dtypes.bfloat16)

    in_maps = []
    for b in range(NCORES):
        # xT tile layout [P, NE, S]: [p, e, i] = x[b][i, e*P + p]
        xT = np.ascontiguousarray(
            x[b].T.reshape(NE, P, S).transpose(1, 0, 2))
        xhi_h, xlo_h = _split8(xT)
        in_maps.append({
            "xhi": xhi_h, "xlo": xlo_h, "ahi": ahi_h, "alo": alo_h,
            "wvh": wvh_h, "wvl": wvl_h, "uw": uw_h,
            "bv": bv_h, "cc": cc_h,
        })
    return in_maps


def kernel(**inputs):
    from concourse.bass_utils import run_bass_kernel_spmd

    nc = _get_built()
    in_maps = _make_in_maps(inputs)
    res = run_bass_kernel_spmd(nc, in_maps, list(range(NCORES)))
    out = np.stack([np.asarray(res.results[b]["out"], dtype=np.float32)
                    for b in range(NCORES)])
    return out
